# revision 15
# baseline (speedup 1.0000x reference)
"""Trainium2 Bass kernel for MartingaleAwareAttention.

Math: the reference runs standard attention plus 20 permutation passes.
Full bidirectional attention with per-token projections is permutation-
equivariant, so each pass (permute -> attend -> unpermute) equals standard
attention exactly in real arithmetic. Hence

    out = (1-a)*std + a*var_w*perm_out = c * (attend(x) @ wo + bo),
    c = (1-a) + a*var_w,  a = clip(len_w * log(S)/S, 0.01, 1.0)

whenever perms are true permutations (checked at runtime; exact numpy
fallback otherwise).

Sharding: 8 cores = 4 batches x 2 head-halves (8 heads / 512 cols each).
Each core computes its half's attention and the partial @ wo[rows] product;
host sums the two halves, adds bo, scales by c.

Device pipeline per core (all matmuls accumulate fp32 in PSUM):
  - inputs x^T | wq | wk | wv packed per 128-row d-chunk into one bf16
    "chunk" tensor each (section-ordered DMAs so Q/K projections start
    as soon as their sections land); wo kept separate in fp32r.
  - Q^T/K^T = wq/wk-chunk^T @ x^T-chunk  (bf16, accumulated over 8 chunks)
  - V_aug[s, h*65+c] = V with a ones column appended per head
  - per head: S^T = K^T_h-slice^T @ Q^T_h -> exp (ACT, scale=1/8) ->
    O^T accum = V_aug_h^T @ P^T  giving [65, 512]: rows 0-63 = unnorm
    O^T, row 64 = softmax denominator
  - denom row -> partition 0 via SBUF-SBUF DMA, reciprocal_approx_fast,
    K=1 ones-matmul broadcast to 64 partitions, multiply (all fp32/f32r)
  - partial = Onorm^T-chunks @ wo-rows (f32r), DMA out
"""

import math

import numpy as np

B = 4
S = 512
DM = 1024
NHL = 8        # local heads per core
HD = 64
JL = NHL * HD  # 512 local head-dim columns
SCALE = HD ** -0.5
NCORES = 8
CW = 2560      # bf16 chunk width: xT|wq|wk|wv (2048) + extras

_PROG = None


def _build_program():
    import concourse.bacc as bacc
    import concourse.tile as tile
    from concourse import mybir

    f32 = mybir.dt.float32
    f32r = mybir.dt.float32r
    bf16 = mybir.dt.bfloat16
    EXP = mybir.ActivationFunctionType.Exp
    COPY = mybir.ActivationFunctionType.Copy

    nc = bacc.Bacc()

    chunks = [
        nc.declare_dram_parameter(f"chunk{dc}", [128, CW], bf16, isOutput=False)
        for dc in range(8)
    ]
    wop = [
        nc.declare_dram_parameter(f"wo{jc}", [128, DM], f32r, isOutput=False)
        for jc in range(4)
    ]
    onehot = nc.declare_dram_parameter("onehot", [NHL, S], f32r,
                                       isOutput=False)
    out = nc.declare_dram_parameter("out", [S, DM], f32, isOutput=True)

    with tile.TileContext(nc) as tc:
        from contextlib import ExitStack

        with ExitStack() as ctx:
            wts = ctx.enter_context(tc.tile_pool(name="wts", bufs=1))
            ppt = ctx.enter_context(tc.tile_pool(name="ppt", bufs=8))

            # ------------- input DMA, section-ordered -------------
            ch = []
            for dc in range(8):
                t = wts.tile([128, CW], bf16, tag=f"ch{dc}", name=f"ch{dc}")
                ch.append(t)
            # xT + wq + wk sections first (scores path), then wv+extras
            for dc in range(8):
                nc.sync.dma_start(out=ch[dc][:, 0:1536],
                                  in_=chunks[dc][:, 0:1536])
            for dc in range(8):
                nc.sync.dma_start(out=ch[dc][:, 1536:CW],
                                  in_=chunks[dc][:, 1536:CW])
            oneh_sb = wts.tile([NHL, S], f32r, tag="oneh", name="oneh_sb")
            nc.sync.dma_start(out=oneh_sb, in_=onehot[:, :])
            wo_sb = []
            for jc in range(4):
                t = wts.tile([128, DM], f32r, tag=f"wo{jc}", name=f"wosb{jc}")
                nc.sync.dma_start(out=t, in_=wop[jc][:, :])
                wo_sb.append(t)

            def xT(dc):
                return ch[dc][:, 0:512]

            def wqc(dc):
                return ch[dc][:, 512:1024]

            def wkc(dc):
                return ch[dc][:, 1024:1536]

            def wvc(dc):
                return ch[dc][:, 1536:2048]

            bvb = ch[4][:, 2048:2560]
            # tensor_scalar needs an fp32 scalar operand; upcast the tiny
            # bias columns once
            bqr = wts.tile([128, 4], f32, tag="bqr", name="bqr")
            nc.vector.tensor_copy(bqr, ch[6][:, 2048:2052])
            bkr = wts.tile([128, 4], f32, tag="bkr", name="bkr")
            nc.vector.tensor_copy(bkr, ch[6][:, 2052:2056])

            # ------------- projections: QT+KT first, V second -------------
            QT = []
            KT = []
            V = []
            for st in range(4):
                t = wts.tile([128, 8 * 65], bf16, tag=f"V{st}", name=f"V{st}")
                nc.vector.memset(t, 1.0)
                V.append(t)
            with tc.tile_pool(name="psP", bufs=1, space="PSUM") as psP:
                psq = []
                psk = []
                for jt in range(4):
                    psq.append(psP.tile([128, S], f32, tag="pa", bufs=8,
                                        name=f"ps_qt{jt}"))
                for jt in range(4):
                    psk.append(psP.tile([128, S], f32, tag="pa", bufs=8,
                                        name=f"ps_kt{jt}"))
                for dc in range(8):
                    for jt in range(4):
                        nc.tensor.matmul(
                            psq[jt],
                            wqc(dc)[:, jt * 128:(jt + 1) * 128],
                            xT(dc),
                            start=(dc == 0), stop=(dc == 7),
                        )
                    for jt in range(4):
                        nc.tensor.matmul(
                            psk[jt],
                            wkc(dc)[:, jt * 128:(jt + 1) * 128],
                            xT(dc),
                            start=(dc == 0), stop=(dc == 7),
                        )
                for jt in range(4):
                    t = wts.tile([128, S], bf16, tag=f"QT{jt}", name=f"QT{jt}")
                    nc.vector.tensor_scalar_add(t, psq[jt], bqr[:, jt:jt + 1])
                    QT.append(t)
                for jt in range(4):
                    t = wts.tile([128, S], bf16, tag=f"KT{jt}", name=f"KT{jt}")
                    nc.vector.tensor_scalar_add(t, psk[jt], bkr[:, jt:jt + 1])
                    KT.append(t)
                # V group reuses freed slots
                psv = []
                for st in range(4):
                    psv.append(psP.tile([128, JL], f32, tag="pa", bufs=8,
                                        name=f"ps_v{st}"))
                for dc in range(8):
                    for st in range(4):
                        nc.tensor.matmul(
                            psv[st],
                            xT(dc)[:, st * 128:(st + 1) * 128],
                            wvc(dc),
                            start=(dc == 0), stop=(dc == 7),
                        )
                for st in range(4):
                    nc.vector.tensor_add(
                        V[st].rearrange("p (h c) -> p h c", c=65)[:, :, 0:64],
                        psv[st].rearrange("p (h c) -> p h c", c=64),
                        bvb.rearrange("p (h c) -> p h c", c=64),
                    )

            # ------------- attention -------------
            # Pass 1 streams both heads of each pair on PE; the recip/DMA
            # latency chain runs on DVE/DMA off the PE program order.
            Onorm = []
            for pair in range(4):
                t = wts.tile([128, S], f32r, tag=f"On{pair}", name=f"On{pair}")
                Onorm.append(t)
            ous = []
            denall = wts.tile([NHL, S], f32, tag="denall", name="denall")
            with tc.tile_pool(name="psA", bufs=1, space="PSUM") as psA:
                for pair in range(4):
                    po = []
                    for hh in range(2):
                        po.append(psA.tile([65, S], f32, tag="o", bufs=2,
                                           name=f"ps_o{2 * pair + hh}"))
                    for kt in range(4):
                        pts = []
                        for hh in range(2):
                            h = 2 * pair + hh
                            base = hh * 64
                            ps_s = psA.tile([128, S], f32, tag="sc", bufs=4,
                                            name=f"ps_s{h}_{kt}")
                            nc.tensor.matmul(
                                ps_s,
                                KT[pair][base:base + 64,
                                         kt * 128:(kt + 1) * 128],
                                QT[pair][base:base + 64, :],
                                start=True, stop=True,
                            )
                            pt = ppt.tile([128, S], bf16, tag="pt", bufs=8,
                                          name=f"pt{h}_{kt}")
                            nc.scalar.activation(pt, ps_s, EXP, scale=SCALE)
                            pts.append(pt)
                        for hh in range(2):
                            h = 2 * pair + hh
                            nc.tensor.matmul(
                                po[hh],
                                V[kt][:, h * 65:(h + 1) * 65],
                                pts[hh],
                                start=(kt == 0), stop=(kt == 3),
                            )
                    for hh in range(2):
                        h = 2 * pair + hh
                        ou = wts.tile([65, S], f32, tag=f"ou{h}", bufs=1,
                                      name=f"ou{h}")
                        nc.vector.tensor_copy(ou, po[hh])
                        ous.append(ou)
                        nc.sync.dma_start(out=denall[h:h + 1, :],
                                          in_=ou[64:65, :])

                # pass 2: batched reciprocal, per-head broadcast + normalize
                rdall = wts.tile([NHL, S], f32, tag="rdall", name="rdall")
                nc.vector.reciprocal_approx_fast(rdall, denall)
                rdr_all = wts.tile([NHL, S], f32r, tag="rdr_all",
                                   name="rdr_all")
                nc.vector.tensor_copy(rdr_all, rdall)
                for h in range(8):
                    pair, hh = divmod(h, 2)
                    ps_r = psA.tile([64, S], f32, tag="r", bufs=2,
                                    name=f"ps_r{h}")
                    nc.tensor.matmul(
                        ps_r,
                        oneh_sb[:, h * 64:(h + 1) * 64],
                        rdr_all,
                        start=True, stop=True,
                    )
                    rsb = wts.tile([64, S], f32, tag="rsb", bufs=2,
                                   name=f"rsb{h}")
                    nc.vector.tensor_copy(rsb, ps_r)
                    if hh == 0:
                        nc.vector.tensor_mul(
                            Onorm[pair][0:64, :], rsb, ous[h][0:64, :])
                    else:
                        tmp = wts.tile([64, S], f32r, tag="tmpon", bufs=2,
                                       name=f"tmpon{h}")
                        nc.vector.tensor_mul(tmp, rsb, ous[h][0:64, :])
                        nc.sync.dma_start(
                            out=Onorm[pair][64:128, :], in_=tmp)

            # ------------- output projection -------------
            with tc.tile_pool(name="psW", bufs=1, space="PSUM") as psW:
                for st in range(4):
                    for mt in range(2):
                        ps_w = psW.tile([128, 512], f32, tag="wop", bufs=2,
                                        name=f"ps_w{st}_{mt}")
                        for jc in range(4):
                            nc.tensor.matmul(
                                ps_w,
                                Onorm[jc][:, st * 128:(st + 1) * 128],
                                wo_sb[jc][:, mt * 512:(mt + 1) * 512],
                                start=(jc == 0), stop=(jc == 3),
                            )
                        osb = wts.tile([128, 512], f32, tag="osb", bufs=3,
                                       name=f"osb{st}_{mt}")
                        nc.vector.tensor_copy(osb, ps_w)
                        nc.sync.dma_start(
                            out=out[st * 128:(st + 1) * 128,
                                    mt * 512:(mt + 1) * 512],
                            in_=osb)

    nc.compile()
    return nc


def _get_prog():
    global _PROG
    if _PROG is None:
        _PROG = _build_program()
    return _PROG


def _pack_chunks(xb, wq_s, wk_s, wv_s, bq_s, bk_s, bv_s, bf16):
    """Build the 8 [128, CW] bf16 chunk arrays for one core."""
    xT = np.ascontiguousarray(xb.T)          # [1024, 512]
    chunks = []
    for dc in range(8):
        c = np.zeros((128, CW), np.float32)
        rs = slice(dc * 128, (dc + 1) * 128)
        c[:, 0:512] = xT[rs]
        c[:, 512:1024] = wq_s[rs]
        c[:, 1024:1536] = wk_s[rs]
        c[:, 1536:2048] = wv_s[rs]
        if dc == 4:
            c[:, 2048:2560] = bv_s[None, :]
        elif dc == 6:
            c[:, 2048:2052] = bq_s.reshape(4, 128).T
            c[:, 2052:2056] = bk_s.reshape(4, 128).T
        chunks.append(c.astype(bf16))
    return chunks


def _attend_np(x, wq, bq, wk, bk, wv, bv):
    Bn, Sn, D = x.shape
    H = D // HD

    def proj(w, b):
        return (x @ w + b).reshape(Bn, Sn, H, HD).transpose(0, 2, 1, 3)

    q, k, v = proj(wq, bq), proj(wk, bk), proj(wv, bv)
    s = np.einsum('bhqd,bhkd->bhqk', q, k) * (HD ** -0.5)
    s = s - s.max(axis=-1, keepdims=True)
    e = np.exp(s)
    attn = e / e.sum(axis=-1, keepdims=True)
    o = np.einsum('bhqk,bhkd->bhqd', attn, v)
    return o.transpose(0, 2, 1, 3).reshape(Bn, Sn, D)


def _numpy_fallback(x, wq, bq, wk, bk, wv, bv, wo, bo, var_w, len_w, perms):
    Sn = x.shape[1]
    standard = _attend_np(x, wq, bq, wk, bk, wv, bv) @ wo + bo
    acc = np.zeros_like(x)
    for p in perms:
        xp = np.take(x, p, axis=1)
        o = _attend_np(xp, wq, bq, wk, bk, wv, bv)
        inv = np.argsort(p)
        acc = acc + np.take(o, inv, axis=1)
    perm_out = (acc / perms.shape[0]) @ wo + bo
    adaptive = np.clip(len_w * (math.log(Sn) / Sn), 0.01, 1.0).astype(np.float32)
    return ((1.0 - adaptive) * standard + adaptive * var_w * perm_out).astype(
        np.float32)


def kernel(x, wq, bq, wk, bk, wv, bv, wo, bo, var_w, len_w, perms, **_kw):
    x = np.ascontiguousarray(np.asarray(x, dtype=np.float32))
    wq = np.asarray(wq, dtype=np.float32)
    bq = np.asarray(bq, dtype=np.float32)
    wk = np.asarray(wk, dtype=np.float32)
    bk = np.asarray(bk, dtype=np.float32)
    wv = np.asarray(wv, dtype=np.float32)
    bv = np.asarray(bv, dtype=np.float32)
    wo = np.asarray(wo, dtype=np.float32)
    bo = np.asarray(bo, dtype=np.float32)
    var_w = np.asarray(var_w, dtype=np.float32)
    len_w = np.asarray(len_w, dtype=np.float32)
    perms_np = np.asarray(perms)

    Sn = x.shape[1]
    idx = np.arange(Sn)
    if not all(np.array_equal(np.sort(p), idx) for p in perms_np):
        return _numpy_fallback(x, wq, bq, wk, bk, wv, bv, wo, bo,
                               var_w, len_w, perms_np)

    import ml_dtypes
    bf16 = ml_dtypes.bfloat16

    adaptive = np.clip(len_w * (math.log(Sn) / Sn), 0.01, 1.0).astype(np.float32)
    c = float(((1.0 - adaptive) + adaptive * var_w).reshape(-1)[0])

    oneh = np.zeros((NHL, S), np.float32)
    for h in range(NHL):
        oneh[h, h * HD:(h + 1) * HD] = 1.0

    in_maps = []
    for core in range(NCORES):
        b, g = divmod(core, 2)
        cs = slice(g * JL, (g + 1) * JL)
        chunks = _pack_chunks(x[b], wq[:, cs], wk[:, cs], wv[:, cs],
                              bq[cs], bk[cs], bv[cs], bf16)
        m = {f"chunk{dc}": chunks[dc] for dc in range(8)}
        m["onehot"] = oneh
        wo_s = np.ascontiguousarray(wo[cs, :])
        for jc in range(4):
            m[f"wo{jc}"] = np.ascontiguousarray(
                wo_s[jc * 128:(jc + 1) * 128, :])
        in_maps.append(m)

    from concourse.bass_utils import run_bass_kernel_spmd

    nc = _get_prog()
    res = run_bass_kernel_spmd(nc, in_maps, list(range(NCORES)))
    parts = [res.results[i]["out"] for i in range(NCORES)]

    outp = np.empty((B, Sn, DM), np.float32)
    for b in range(B):
        outp[b] = c * (parts[2 * b] + parts[2 * b + 1] + bo[None, :])
    return outp


# revision 17
# speedup vs baseline: 1.0275x; 1.0275x over previous
"""Trainium2 Bass kernel for MartingaleAwareAttention.

Math: the reference runs standard attention plus 20 permutation passes.
Full bidirectional attention with per-token projections is permutation-
equivariant, so each pass (permute -> attend -> unpermute) equals standard
attention exactly in real arithmetic. Hence

    out = (1-a)*std + a*var_w*perm_out = c * (attend(x) @ wo + bo),
    c = (1-a) + a*var_w,  a = clip(len_w * log(S)/S, 0.01, 1.0)

whenever perms are true permutations (checked at runtime; exact numpy
fallback otherwise).

Sharding: 8 cores = 4 batches x 2 head-halves (8 heads / 512 cols each).
Each core computes its half's attention and the partial @ wo[rows] product;
host sums the two halves, adds bo, scales by c.

Device pipeline per core (all matmuls accumulate fp32 in PSUM):
  - inputs x^T | wq | wk | wv packed per 128-row d-chunk into one bf16
    "chunk" tensor each (section-ordered DMAs so Q/K projections start
    as soon as their sections land); wo kept separate in fp32r.
  - Q^T/K^T = wq/wk-chunk^T @ x^T-chunk  (bf16, accumulated over 8 chunks)
  - V_aug[s, h*65+c] = V with a ones column appended per head
  - per head: S^T = K^T_h-slice^T @ Q^T_h -> exp (ACT, scale=1/8) ->
    O^T accum = V_aug_h^T @ P^T  giving [65, 512]: rows 0-63 = unnorm
    O^T, row 64 = softmax denominator
  - denom row -> partition 0 via SBUF-SBUF DMA, reciprocal_approx_fast,
    K=1 ones-matmul broadcast to 64 partitions, multiply (all fp32/f32r)
  - partial = Onorm^T-chunks @ wo-rows (f32r), DMA out
"""

import math

import numpy as np

B = 4
S = 512
DM = 1024
NHL = 8        # local heads per core
HD = 64
JL = NHL * HD  # 512 local head-dim columns
SCALE = HD ** -0.5
NCORES = 8
CW = 2560      # bf16 chunk width: xT|wq|wk|wv (2048) + extras

_PROG = None


def _build_program():
    import concourse.bacc as bacc
    import concourse.tile as tile
    from concourse import mybir
    from concourse.tile import add_dep_helper

    f32 = mybir.dt.float32
    f32r = mybir.dt.float32r
    bf16 = mybir.dt.bfloat16
    EXP = mybir.ActivationFunctionType.Exp
    COPY = mybir.ActivationFunctionType.Copy

    nc = bacc.Bacc()

    chunks = [
        nc.declare_dram_parameter(f"chunk{dc}", [128, CW], bf16, isOutput=False)
        for dc in range(8)
    ]
    wop = [
        nc.declare_dram_parameter(f"wo{jc}", [128, DM], f32r, isOutput=False)
        for jc in range(4)
    ]
    onehot = nc.declare_dram_parameter("onehot", [NHL, S], f32r,
                                       isOutput=False)
    out = nc.declare_dram_parameter("out", [S, DM], f32, isOutput=True)

    with tile.TileContext(nc) as tc:
        from contextlib import ExitStack

        with ExitStack() as ctx:
            wts = ctx.enter_context(tc.tile_pool(name="wts", bufs=1))
            ppt = ctx.enter_context(tc.tile_pool(name="ppt", bufs=8))

            # ------------- input DMA, section-ordered -------------
            ch = []
            for dc in range(8):
                t = wts.tile([128, CW], bf16, tag=f"ch{dc}", name=f"ch{dc}")
                ch.append(t)
            # xT + wq + wk sections first (scores path); wv wave and wo
            # deferred behind wave 1 so they don't steal DMA bandwidth
            # from the critical path.
            d1 = []
            for dc in range(8):
                d1.append(nc.sync.dma_start(out=ch[dc][:, 0:1536],
                                            in_=chunks[dc][:, 0:1536]))
            d2 = []
            for dc in range(8):
                i2 = nc.sync.dma_start(out=ch[dc][:, 1536:CW],
                                       in_=chunks[dc][:, 1536:CW])
                add_dep_helper(i2.ins, d1[dc].ins, reason="defer wv")
                d2.append(i2)
            oneh_sb = wts.tile([NHL, S], f32r, tag="oneh", name="oneh_sb")
            nc.sync.dma_start(out=oneh_sb, in_=onehot[:, :])
            wo_sb = []
            for jc in range(4):
                t = wts.tile([128, DM], f32r, tag=f"wo{jc}", name=f"wosb{jc}")
                iw = nc.sync.dma_start(out=t, in_=wop[jc][:, :])
                add_dep_helper(iw.ins, d2[2 * jc].ins, reason="defer wo")
                add_dep_helper(iw.ins, d2[2 * jc + 1].ins, reason="defer wo")
                wo_sb.append(t)

            def xT(dc):
                return ch[dc][:, 0:512]

            def wqc(dc):
                return ch[dc][:, 512:1024]

            def wkc(dc):
                return ch[dc][:, 1024:1536]

            def wvc(dc):
                return ch[dc][:, 1536:2048]

            bvb = ch[4][:, 2048:2560]
            # tensor_scalar needs an fp32 scalar operand; upcast the tiny
            # bias columns once
            bqr = wts.tile([128, 4], f32, tag="bqr", name="bqr")
            nc.vector.tensor_copy(bqr, ch[6][:, 2048:2052])
            bkr = wts.tile([128, 4], f32, tag="bkr", name="bkr")
            nc.vector.tensor_copy(bkr, ch[6][:, 2052:2056])

            # ------------- projections: QT+KT first, V second -------------
            QT = []
            KT = []
            V = []
            for st in range(4):
                t = wts.tile([128, 8 * 65], bf16, tag=f"V{st}", name=f"V{st}")
                nc.vector.memset(t, 1.0)
                V.append(t)
            with tc.tile_pool(name="psP", bufs=1, space="PSUM") as psP:
                psq = []
                psk = []
                for jt in range(4):
                    psq.append(psP.tile([128, S], f32, tag="pa", bufs=8,
                                        name=f"ps_qt{jt}"))
                for jt in range(4):
                    psk.append(psP.tile([128, S], f32, tag="pa", bufs=8,
                                        name=f"ps_kt{jt}"))
                for dc in range(8):
                    for jt in range(4):
                        nc.tensor.matmul(
                            psq[jt],
                            wqc(dc)[:, jt * 128:(jt + 1) * 128],
                            xT(dc),
                            start=(dc == 0), stop=(dc == 7),
                        )
                    for jt in range(4):
                        nc.tensor.matmul(
                            psk[jt],
                            wkc(dc)[:, jt * 128:(jt + 1) * 128],
                            xT(dc),
                            start=(dc == 0), stop=(dc == 7),
                        )
                for jt in range(4):
                    t = wts.tile([128, S], bf16, tag=f"QT{jt}", name=f"QT{jt}")
                    nc.vector.tensor_scalar_add(t, psq[jt], bqr[:, jt:jt + 1])
                    QT.append(t)
                for jt in range(4):
                    t = wts.tile([128, S], bf16, tag=f"KT{jt}", name=f"KT{jt}")
                    nc.vector.tensor_scalar_add(t, psk[jt], bkr[:, jt:jt + 1])
                    KT.append(t)
                # V group reuses freed slots
                psv = []
                for st in range(4):
                    psv.append(psP.tile([128, JL], f32, tag="pa", bufs=8,
                                        name=f"ps_v{st}"))
                for dc in range(8):
                    for st in range(4):
                        nc.tensor.matmul(
                            psv[st],
                            xT(dc)[:, st * 128:(st + 1) * 128],
                            wvc(dc),
                            start=(dc == 0), stop=(dc == 7),
                        )
                for st in range(4):
                    nc.vector.tensor_add(
                        V[st].rearrange("p (h c) -> p h c", c=65)[:, :, 0:64],
                        psv[st].rearrange("p (h c) -> p h c", c=64),
                        bvb.rearrange("p (h c) -> p h c", c=64),
                    )

            # ------------- attention -------------
            # Pass 1 streams both heads of each pair on PE; the recip/DMA
            # latency chain runs on DVE/DMA off the PE program order.
            Onorm = []
            for pair in range(4):
                t = wts.tile([128, S], f32r, tag=f"On{pair}", name=f"On{pair}")
                Onorm.append(t)
            ous = []
            denall = wts.tile([NHL, S], f32, tag="denall", name="denall")
            with tc.tile_pool(name="psA", bufs=1, space="PSUM") as psA:
                for pair in range(4):
                    po = []
                    for hh in range(2):
                        po.append(psA.tile([65, S], f32, tag="o", bufs=2,
                                           name=f"ps_o{2 * pair + hh}"))

                    def emit_score(kt, hh, pair=pair):
                        h = 2 * pair + hh
                        base = hh * 64
                        ps_s = psA.tile([128, S], f32, tag="sc", bufs=4,
                                        name=f"ps_s{h}_{kt}")
                        nc.tensor.matmul(
                            ps_s,
                            KT[pair][base:base + 64,
                                     kt * 128:(kt + 1) * 128],
                            QT[pair][base:base + 64, :],
                            start=True, stop=True,
                        )
                        pt = ppt.tile([128, S], bf16, tag="pt", bufs=8,
                                      name=f"pt{h}_{kt}")
                        nc.scalar.activation(pt, ps_s, EXP, scale=SCALE)
                        return pt

                    pts = {}
                    for hh in range(2):
                        pts[(0, hh)] = emit_score(0, hh)
                    for hh in range(2):
                        pts[(1, hh)] = emit_score(1, hh)
                    for kt in range(4):
                        for hh in range(2):
                            h = 2 * pair + hh
                            nc.tensor.matmul(
                                po[hh],
                                V[kt][:, h * 65:(h + 1) * 65],
                                pts.pop((kt, hh)),
                                start=(kt == 0), stop=(kt == 3),
                            )
                        if kt + 2 <= 3:
                            for hh in range(2):
                                pts[(kt + 2, hh)] = emit_score(kt + 2, hh)
                    for hh in range(2):
                        h = 2 * pair + hh
                        ou = wts.tile([65, S], f32, tag=f"ou{h}", bufs=1,
                                      name=f"ou{h}")
                        nc.vector.tensor_copy(ou, po[hh])
                        ous.append(ou)
                        nc.sync.dma_start(out=denall[h:h + 1, :],
                                          in_=ou[64:65, :])

                # pass 2: batched reciprocal, per-head broadcast + normalize
                rdall = wts.tile([NHL, S], f32, tag="rdall", name="rdall")
                nc.vector.reciprocal_approx_fast(rdall, denall)
                rdr_all = wts.tile([NHL, S], f32r, tag="rdr_all",
                                   name="rdr_all")
                nc.vector.tensor_copy(rdr_all, rdall)
                for h in range(8):
                    pair, hh = divmod(h, 2)
                    ps_r = psA.tile([64, S], f32, tag="r", bufs=2,
                                    name=f"ps_r{h}")
                    nc.tensor.matmul(
                        ps_r,
                        oneh_sb[:, h * 64:(h + 1) * 64],
                        rdr_all,
                        start=True, stop=True,
                    )
                    rsb = wts.tile([64, S], f32, tag="rsb", bufs=2,
                                   name=f"rsb{h}")
                    nc.scalar.activation(rsb, ps_r, COPY)
                    if hh == 0:
                        nc.vector.tensor_mul(
                            Onorm[pair][0:64, :], rsb, ous[h][0:64, :])
                    else:
                        tmp = wts.tile([64, S], f32r, tag="tmpon", bufs=2,
                                       name=f"tmpon{h}")
                        nc.vector.tensor_mul(tmp, rsb, ous[h][0:64, :])
                        nc.sync.dma_start(
                            out=Onorm[pair][64:128, :], in_=tmp)

            # ------------- output projection -------------
            with tc.tile_pool(name="psW", bufs=1, space="PSUM") as psW:
                for st in range(4):
                    for mt in range(2):
                        ps_w = psW.tile([128, 512], f32, tag="wop", bufs=2,
                                        name=f"ps_w{st}_{mt}")
                        for jc in range(4):
                            nc.tensor.matmul(
                                ps_w,
                                Onorm[jc][:, st * 128:(st + 1) * 128],
                                wo_sb[jc][:, mt * 512:(mt + 1) * 512],
                                start=(jc == 0), stop=(jc == 3),
                            )
                        osb = wts.tile([128, 512], f32, tag="osb", bufs=3,
                                       name=f"osb{st}_{mt}")
                        nc.scalar.activation(osb, ps_w, COPY)
                        nc.sync.dma_start(
                            out=out[st * 128:(st + 1) * 128,
                                    mt * 512:(mt + 1) * 512],
                            in_=osb)

    nc.compile()
    return nc


def _get_prog():
    global _PROG
    if _PROG is None:
        _PROG = _build_program()
    return _PROG


def _pack_chunks(xb, wq_s, wk_s, wv_s, bq_s, bk_s, bv_s, bf16):
    """Build the 8 [128, CW] bf16 chunk arrays for one core."""
    xT = np.ascontiguousarray(xb.T)          # [1024, 512]
    chunks = []
    for dc in range(8):
        c = np.zeros((128, CW), np.float32)
        rs = slice(dc * 128, (dc + 1) * 128)
        c[:, 0:512] = xT[rs]
        c[:, 512:1024] = wq_s[rs]
        c[:, 1024:1536] = wk_s[rs]
        c[:, 1536:2048] = wv_s[rs]
        if dc == 4:
            c[:, 2048:2560] = bv_s[None, :]
        elif dc == 6:
            c[:, 2048:2052] = bq_s.reshape(4, 128).T
            c[:, 2052:2056] = bk_s.reshape(4, 128).T
        chunks.append(c.astype(bf16))
    return chunks


def _attend_np(x, wq, bq, wk, bk, wv, bv):
    Bn, Sn, D = x.shape
    H = D // HD

    def proj(w, b):
        return (x @ w + b).reshape(Bn, Sn, H, HD).transpose(0, 2, 1, 3)

    q, k, v = proj(wq, bq), proj(wk, bk), proj(wv, bv)
    s = np.einsum('bhqd,bhkd->bhqk', q, k) * (HD ** -0.5)
    s = s - s.max(axis=-1, keepdims=True)
    e = np.exp(s)
    attn = e / e.sum(axis=-1, keepdims=True)
    o = np.einsum('bhqk,bhkd->bhqd', attn, v)
    return o.transpose(0, 2, 1, 3).reshape(Bn, Sn, D)


def _numpy_fallback(x, wq, bq, wk, bk, wv, bv, wo, bo, var_w, len_w, perms):
    Sn = x.shape[1]
    standard = _attend_np(x, wq, bq, wk, bk, wv, bv) @ wo + bo
    acc = np.zeros_like(x)
    for p in perms:
        xp = np.take(x, p, axis=1)
        o = _attend_np(xp, wq, bq, wk, bk, wv, bv)
        inv = np.argsort(p)
        acc = acc + np.take(o, inv, axis=1)
    perm_out = (acc / perms.shape[0]) @ wo + bo
    adaptive = np.clip(len_w * (math.log(Sn) / Sn), 0.01, 1.0).astype(np.float32)
    return ((1.0 - adaptive) * standard + adaptive * var_w * perm_out).astype(
        np.float32)


def kernel(x, wq, bq, wk, bk, wv, bv, wo, bo, var_w, len_w, perms, **_kw):
    x = np.ascontiguousarray(np.asarray(x, dtype=np.float32))
    wq = np.asarray(wq, dtype=np.float32)
    bq = np.asarray(bq, dtype=np.float32)
    wk = np.asarray(wk, dtype=np.float32)
    bk = np.asarray(bk, dtype=np.float32)
    wv = np.asarray(wv, dtype=np.float32)
    bv = np.asarray(bv, dtype=np.float32)
    wo = np.asarray(wo, dtype=np.float32)
    bo = np.asarray(bo, dtype=np.float32)
    var_w = np.asarray(var_w, dtype=np.float32)
    len_w = np.asarray(len_w, dtype=np.float32)
    perms_np = np.asarray(perms)

    Sn = x.shape[1]
    idx = np.arange(Sn)
    if not all(np.array_equal(np.sort(p), idx) for p in perms_np):
        return _numpy_fallback(x, wq, bq, wk, bk, wv, bv, wo, bo,
                               var_w, len_w, perms_np)

    import ml_dtypes
    bf16 = ml_dtypes.bfloat16

    adaptive = np.clip(len_w * (math.log(Sn) / Sn), 0.01, 1.0).astype(np.float32)
    c = float(((1.0 - adaptive) + adaptive * var_w).reshape(-1)[0])

    oneh = np.zeros((NHL, S), np.float32)
    for h in range(NHL):
        oneh[h, h * HD:(h + 1) * HD] = 1.0

    in_maps = []
    for core in range(NCORES):
        b, g = divmod(core, 2)
        cs = slice(g * JL, (g + 1) * JL)
        chunks = _pack_chunks(x[b], wq[:, cs], wk[:, cs], wv[:, cs],
                              bq[cs], bk[cs], bv[cs], bf16)
        m = {f"chunk{dc}": chunks[dc] for dc in range(8)}
        m["onehot"] = oneh
        wo_s = np.ascontiguousarray(wo[cs, :])
        for jc in range(4):
            m[f"wo{jc}"] = np.ascontiguousarray(
                wo_s[jc * 128:(jc + 1) * 128, :])
        in_maps.append(m)

    from concourse.bass_utils import run_bass_kernel_spmd

    nc = _get_prog()
    res = run_bass_kernel_spmd(nc, in_maps, list(range(NCORES)))
    parts = [res.results[i]["out"] for i in range(NCORES)]

    outp = np.empty((B, Sn, DM), np.float32)
    for b in range(B):
        outp[b] = c * (parts[2 * b] + parts[2 * b + 1] + bo[None, :])
    return outp
